# revision 34
# baseline (speedup 1.0000x reference)
"""Trainium2 Bass kernel for 16-head MultiHeadAttention (B=2, S=2048, D=1024).

Sharding: 8 cores = 2 (batch) x 4 (head groups of 4 heads).
Each core computes qkv projection for its 4 heads, attention, and a partial
out-projection (TP over heads); host sums the 4 partials per batch element.

Host-side sharding marshals data into device-friendly layouts (x and weights
pre-transposed so the contraction dim lands on SBUF partitions; biases
pre-stacked). On-device pipeline (single dependency-streamed pass):
  - Q^T(first half)/K^T(first half) projected as soon as their x^T s-blocks
    arrive; softmax exp (the ACT-engine floor, ~133us) starts ~18us in.
  - Attention per (q-half, head): scores -> PSUM ping-pong, exp [128,1024]
    on ACT, PV in transposed orientation (out [q=128, dk+1]) with the softmax
    denominator via a ones-column in V'. PV of head h overlaps exp of h+1;
    remaining projections/PV/out-proj are slot-woven into the score streams
    to fill PE slack under the exp stream.
  - Normalization via per-partition reciprocal+tensor_scalar_mul; attention
    rows stream into at^T (DMA transpose) + out-projection + DMA per s-block.
"""

import sys
from dataclasses import dataclass

for _p in ("/opt/trn_rl_repo",):
    if _p not in sys.path:
        sys.path.insert(0, _p)

import numpy as np

import concourse.bass as bass  # noqa: E402,F401
import concourse.bacc as bacc  # noqa: E402
import concourse.tile as tile  # noqa: E402
from concourse import mybir  # noqa: E402
from concourse.bass_utils import run_bass_kernel_spmd  # noqa: E402

F32 = mybir.dt.float32
BF16 = mybir.dt.bfloat16
AF = mybir.ActivationFunctionType


@dataclass(frozen=True)
class Cfg:
    S: int = 2048      # sequence length
    DIN: int = 1024    # model dim
    HPC: int = 4       # heads per core
    DK: int = 64       # head dim
    N_CORES: int = 8

    @property
    def DQK(self):
        return self.HPC * self.DK  # 256 per-core q/k/v width

    @property
    def KC(self):
        return self.DIN // 128     # 8 contraction chunks

    @property
    def SB(self):
        return self.S // 128       # 16 sequence blocks


FULL = Cfg()


def build_nc(cfg: Cfg = FULL):
    S, DIN, HPC, DK = cfg.S, cfg.DIN, cfg.HPC, cfg.DK
    DQK, KC, SB = cfg.DQK, cfg.KC, cfg.SB
    QC = 1024                 # q-chunk width for attention passes
    SBH = QC // 128           # 8 s-blocks per pass
    SCALE_INV = 1.0 / float(np.sqrt(DK))
    V65 = DK + 1              # V' width per head (denominator ones col)

    nc = bacc.Bacc("TRN2", target_bir_lowering=False, debug=False,
                   num_devices=cfg.N_CORES)

    # x^T in bf16, delivered as 2-s-block slabs: row k2*128+p holds
    # x[k2*256+j, c*128+p] at col c*256+j (see shard_inputs) so each
    # [128,2048] DMA delivers all 8 c-chunks of two s-blocks contiguously.
    xt_d = nc.dram_tensor("xt", [8 * 128, S], BF16, kind="ExternalInput")
    wqkT_d = nc.dram_tensor("w_qkT", [128, 4 * DIN], BF16,
                            kind="ExternalInput")
    wvT_d = nc.dram_tensor("w_vT", [128, KC * DQK], BF16,
                           kind="ExternalInput")
    woT_d = nc.dram_tensor("w_oT", [128, 2 * DIN], BF16,
                           kind="ExternalInput")
    bqk_d = nc.dram_tensor("b_qk", [128, 4], F32, kind="ExternalInput")
    id_d = nc.dram_tensor("ident", [128, 128], BF16, kind="ExternalInput")
    bv_d = nc.dram_tensor("b_v128", [128, DQK], F32, kind="ExternalInput")
    bo_d = nc.dram_tensor("b_o128", [128, DIN], F32, kind="ExternalInput")
    out_d = nc.dram_tensor("out_partial", [S, DIN], BF16,
                           kind="ExternalOutput")

    with tile.TileContext(nc) as tc:
        with (
            tc.tile_pool(name="persist", bufs=1) as pp,
            tc.tile_pool(name="expp", bufs=48) as ep,         # exp outputs
            tc.tile_pool(name="recp", bufs=4) as rp,
            tc.tile_pool(name="outp", bufs=2) as op_,
            tc.tile_pool(name="ps_s", bufs=2, space="PSUM") as pss,
            tc.tile_pool(name="ps_v", bufs=2, space="PSUM") as ppv,
            tc.tile_pool(name="ps_o", bufs=2, space="PSUM") as pso,
        ):
            # ---- persistent SBUF tensors (all-bf16 data path) ----
            xt = pp.tile([128, KC * S], BF16, tag="xt")          # x^T  [c][s]
            wqkT = pp.tile([128, 4 * DIN], BF16, tag="wqkT")     # [blk][c][j]
            wvT = pp.tile([128, KC * DQK], BF16, tag="wvT")      # [c][dout]
            woT = pp.tile([128, 2 * DIN], BF16, tag="woT")       # [ch][dm]
            qk = pp.tile([128, 4 * S], BF16, tag="qk")           # q0,q1,k0,k1
            vv = pp.tile([128, SB * HPC * V65], BF16, tag="vv")  # V' blocks
            at = pp.tile([128, SB * DQK], BF16, tag="at")        # attn out
            atT = pp.tile([128, 2 * S], BF16, tag="atT")         # at^T
            bqk = pp.tile([128, 4], F32, tag="bqk")
            ident = pp.tile([128, 128], BF16, tag="ident")
            bv128 = pp.tile([128, DQK], F32, tag="bv128")
            bo128 = pp.tile([128, DIN], F32, tag="bo128")

            xt3 = xt[:].rearrange("p (c s) -> p c s", c=KC)
            wqkT3 = wqkT[:].rearrange("p (b c j) -> p b c j", b=4, c=KC)
            wvT3 = wvT[:].rearrange("p (c d) -> p c d", c=KC)
            woT3 = woT[:].rearrange("p (h d) -> p h d", h=2)
            qk3 = qk[:].rearrange("p (b s) -> p b s", b=4)
            vv4 = vv[:].rearrange("p (i h d) -> p i h d", i=SB, h=HPC)
            at3 = at[:].rearrange("p (i d) -> p i d", i=SB)
            atT3 = atT[:].rearrange("p (h s) -> p h s", h=2)

            # ---- PE warm-up: keep the tensor engine busy from t=0 so
            # the p-state ramp completes before the first real projection ----
            warm = pp.tile([128, 512], BF16, tag="warm")
            nc.vector.memset(warm[:], 0.0)
            for _ in range(16):
                psw = pss.tile([128, QC], F32, tag="pss")
                nc.tensor.matmul(psw[:, 0:512], warm[:, 0:128], warm[:],
                                 start=True, stop=True)

            # ---- small loads ----
            for i in range(SB):
                nc.vector.memset(vv4[:, i, :, DK:V65], 1.0)
            # (bv128/bo128 loaded after the bulk weights/x below)

            def pe_keepwarm(n):
                # dep-free matmuls that fill known PE stall windows so the
                # p-state ramp doesn't reset (post-stall matmuls run 2-3.7x
                # slower in the cost model)
                for _ in range(n):
                    psw = pss.tile([128, QC], F32, tag="pss")
                    nc.tensor.matmul(psw[:, 0:512], warm[:, 0:128], warm[:],
                                     start=True, stop=True)

            # ---- bulk loads, ordered for earliest first score ----
            def dma_xt(k2):
                # two s-blocks per DMA (keeps >=512B contiguity)
                nc.sync.dma_start(
                    xt3[:, :, k2 * 256:(k2 + 1) * 256],
                    xt_d.ap()[k2 * 128:(k2 + 1) * 128, :])

            def dma_wqkT(b):
                nc.sync.dma_start(wqkT[:, b * DIN:(b + 1) * DIN],
                                  wqkT_d.ap()[:, b * DIN:(b + 1) * DIN])

            dma_wqkT(0)
            dma_xt(0)
            dma_xt(1)
            nc.sync.dma_start(bqk[:], bqk_d.ap())
            dma_wqkT(2)
            for k2 in range(2, 8):
                dma_xt(k2)
            nc.sync.dma_start(wvT[:], wvT_d.ap())
            dma_wqkT(1)
            dma_wqkT(3)
            nc.sync.dma_start(bv128[:], bv_d.ap())
            nc.sync.dma_start(woT[:], woT_d.ap())
            nc.sync.dma_start(bo128[:], bo_d.ap())
            nc.sync.dma_start(ident[:], id_d.ap())

            # ---- projections (bf16 matmuls, bf16 outputs) ----
            def proj_qkT(dblk, sc):
                # Q^T/K^T block dblk over s columns [sc*512, (sc+1)*512)
                ps = pss.tile([128, QC], F32, tag="pss")
                for c in range(KC):
                    nc.tensor.matmul(
                        ps[:, 0:512],
                        wqkT3[:, dblk, c, :],
                        xt3[:, c, sc * 512:(sc + 1) * 512],
                        start=(c == 0), stop=(c == KC - 1))
                nc.vector.tensor_scalar_add(
                    qk3[:, dblk, sc * 512:(sc + 1) * 512],
                    ps[:, 0:512], bqk[:, dblk:dblk + 1])

            def proj_v(i):
                ps = ppv.tile([128, 512], F32, tag="ppv")
                for c in range(KC):
                    nc.tensor.matmul(
                        ps[:, 0:DQK],
                        xt3[:, c, i * 128:(i + 1) * 128],
                        wvT3[:, c, :],
                        start=(c == 0), stop=(c == KC - 1))
                nc.vector.tensor_add(
                    vv4[:, i, :, 0:DK],
                    ps[:, 0:DQK].rearrange("p (h d) -> p h d", h=HPC),
                    bv128[:].rearrange("p (h d) -> p h d", h=HPC))

            proj_qkT(0, 0)
            proj_qkT(2, 0)
            proj_qkT(0, 1)
            proj_qkT(2, 1)

            # ---- attention machinery ----
            def scores_exp(qc, h, fills):
                """scores+exp for head h over q cols [qc*QC, (qc+1)*QC).
                fills: {slot: [thunks]} woven after score block <slot>."""
                pr, hl = divmod(h, 2)
                qblk, kblk = pr, 2 + pr
                exs = []
                for i in range(SB):
                    ps = pss.tile([128, QC], F32, tag="pss")
                    for n2 in range(QC // 512):
                        nc.tensor.matmul(
                            ps[:, n2 * 512:(n2 + 1) * 512],
                            qk3[64 * hl:64 * hl + 64, kblk,
                                i * 128:(i + 1) * 128],
                            qk3[64 * hl:64 * hl + 64, qblk,
                                qc * QC + n2 * 512:qc * QC + (n2 + 1) * 512],
                            start=True, stop=True)
                    ex = ep.tile([128, QC], BF16, tag="ex")
                    nc.scalar.activation(ex[:], ps[:], AF.Exp,
                                         scale=SCALE_INV)
                    exs.append(ex)
                    for t in fills.get(i, []):
                        t()
                return exs

            def scores_exp_half(qc, h, n2, fills):
                """half-width (512 q cols) scores+exp pass, for splitting the
                final head so its PV/out-proj can start early."""
                pr, hl = divmod(h, 2)
                qblk, kblk = pr, 2 + pr
                exs = []
                for i in range(SB):
                    ps = pss.tile([128, QC], F32, tag="pss")
                    nc.tensor.matmul(
                        ps[:, 0:512],
                        qk3[64 * hl:64 * hl + 64, kblk,
                            i * 128:(i + 1) * 128],
                        qk3[64 * hl:64 * hl + 64, qblk,
                            qc * QC + n2 * 512:qc * QC + (n2 + 1) * 512],
                        start=True, stop=True)
                    ex = ep.tile([128, QC], BF16, tag="ex")
                    nc.scalar.activation(ex[:, 0:512], ps[:, 0:512], AF.Exp,
                                         scale=SCALE_INV)
                    exs.append(ex)
                    for t in fills.get(i, []):
                        t()
                return exs

            def pv_one(qc, h, exs, qb, with_atT=False, half=False,
                       pe_atT=False):
                # PV (transposed) + normalize for one q-block of head h.
                # half: exs are [128,512] half-width tiles; qb is local 0-3.
                sblk = qc * SBH + qb
                qoff = (qb % 4) * 128 if half else qb * 128
                po = ppv.tile([128, 512], F32, tag="ppv")
                for i in range(SB):
                    nc.tensor.matmul(
                        po[:, 0:V65],
                        exs[i][:, qoff:qoff + 128],
                        vv4[:, i, h, :],
                        start=(i == 0), stop=(i == SB - 1))
                rec = rp.tile([128, 1], F32, tag="rec")
                nc.vector.reciprocal(rec[:], po[:, DK:V65])
                nc.vector.tensor_scalar_mul(
                    at3[:, sblk, h * DK:(h + 1) * DK],
                    po[:, 0:DK], rec[:])
                if pe_atT:
                    # PE transpose of the at row-block (no DMA latency)
                    pst = pso.tile([128, 512], F32, tag="pso")
                    pstb = pst.bitcast(BF16)
                    for ch in range(2):
                        nc.tensor.matmul(
                            pstb[:, ch * 128:(ch + 1) * 128],
                            at3[:, sblk, ch * 128:(ch + 1) * 128],
                            ident[:], is_transpose=True,
                            start=(ch == 0), stop=(ch == 1),
                            skip_group_check=True)
                    nc.vector.tensor_copy(
                        atT3[:, :, sblk * 128:(sblk + 1) * 128],
                        pstb[:, 0:256].rearrange("p (h j) -> p h j", h=2))
                elif with_atT:
                    nc.sync.dma_start_transpose(
                        atT3[:, :, sblk * 128:(sblk + 1) * 128],
                        at3[:, sblk, :])

            def outproj_mm(sblk):
                # at^T for sblk must already be in flight (pv_one with_atT)
                ot = op_.tile([128, DIN], BF16, tag="ot")
                for dmh in range(2):
                    ps = pso.tile([128, 512], F32, tag="pso")
                    for ch in range(2):
                        nc.tensor.matmul(
                            ps[:],
                            atT3[:, ch, sblk * 128:(sblk + 1) * 128],
                            woT3[:, ch, dmh * 512:(dmh + 1) * 512],
                            start=(ch == 0), stop=(ch == 1))
                    nc.vector.tensor_add(
                        ot[:, dmh * 512:(dmh + 1) * 512], ps[:],
                        bo128[:, dmh * 512:(dmh + 1) * 512])
                nc.sync.dma_start(
                    out_d.ap()[sblk * 128:(sblk + 1) * 128, :], ot[:])

            def F(*thunks):
                return list(thunks)

            qkp = (lambda d, sc: (lambda: proj_qkT(d, sc)))

            def qkp2(dblk, sc):
                # proj_qkT split into two 4-chunk thunks (finer PE weaving)
                box = []

                def first():
                    ps = pss.tile([128, QC], F32, tag="pss")
                    box.append(ps)
                    for c in range(4):
                        nc.tensor.matmul(
                            ps[:, 0:512],
                            wqkT3[:, dblk, c, :],
                            xt3[:, c, sc * 512:(sc + 1) * 512],
                            start=(c == 0), stop=False)

                def second():
                    ps = box[0]
                    for c in range(4, KC):
                        nc.tensor.matmul(
                            ps[:, 0:512],
                            wqkT3[:, dblk, c, :],
                            xt3[:, c, sc * 512:(sc + 1) * 512],
                            start=False, stop=(c == KC - 1))
                    nc.vector.tensor_scalar_add(
                        qk3[:, dblk, sc * 512:(sc + 1) * 512],
                        ps[:, 0:512], bqk[:, dblk:dblk + 1])
                return first, second
            vp = (lambda i: (lambda: proj_v(i)))

            def pvs(qc, h, exs_ref, opj=False):
                # one thunk per q-block; exs_ref resolved lazily by name
                def mk(qb):
                    def t():
                        pv_one(qc, h, exs_ref(), qb, with_atT=opj)
                        if opj and qb > 0:
                            outproj_mm(qc * SBH + qb - 1)
                    return t
                return [mk(qb) for qb in range(SBH)]

            # qc0
            k22a, k22b = qkp2(2, 2)
            k23a, k23b = qkp2(2, 3)
            ex_h0 = scores_exp(0, 0, {
                4: F(k22a), 5: F(k22b), 7: F(k23a), 8: F(k23b),
                9: F(vp(0)), 10: F(vp(1)), 11: F(vp(2)), 12: F(vp(3)),
                13: F(vp(4)), 14: F(vp(5)), 15: F(vp(6))})
            q10a, q10b = qkp2(1, 0)
            q11a, q11b = qkp2(1, 1)
            k30a, k30b = qkp2(3, 0)
            k31a, k31b = qkp2(3, 1)
            k32a, k32b = qkp2(3, 2)
            k33a, k33b = qkp2(3, 3)
            ex_h1 = scores_exp(0, 1, {
                0: F(q10a), 1: F(q10b), 3: F(q11a), 4: F(q11b),
                6: F(k30a), 7: F(k30b), 9: F(k31a), 10: F(k31b),
                12: F(k32a), 13: F(k32b), 14: F(k33a), 15: F(k33b)})
            pv00 = pvs(0, 0, lambda: ex_h0)
            ex_h2 = scores_exp(0, 2, {
                0: F(vp(7)), 1: F(vp(8)), 2: F(vp(9)), 3: F(vp(10)),
                4: F(vp(11)), 5: F(vp(12)), 6: F(vp(13)), 7: F(vp(14)),
                8: F(vp(15)),
                9: F(pv00[0]), 10: F(pv00[1]), 11: F(pv00[2]),
                12: F(pv00[3]), 13: F(pv00[4]), 14: F(pv00[5]),
                15: F(pv00[6], pv00[7])})
            q02a, q02b = qkp2(0, 2)
            q03a, q03b = qkp2(0, 3)
            pv01 = pvs(0, 1, lambda: ex_h1)
            ex_h3 = scores_exp(0, 3, {
                0: F(q02a), 1: F(q02b), 4: F(q03a), 5: F(q03b),
                2: F(pv01[0]), 3: F(pv01[1]), 6: F(pv01[2]), 7: F(pv01[3]),
                9: F(pv01[4]), 11: F(pv01[5]), 13: F(pv01[6]),
                15: F(pv01[7])})
            # qc1 (finishes qc0's PV + out-projection along the way)
            q12a, q12b = qkp2(1, 2)
            q13a, q13b = qkp2(1, 3)
            pv02 = pvs(0, 2, lambda: ex_h2)
            pe_keepwarm(4)
            ex_g0 = scores_exp(1, 0, {
                0: F(pv02[0]), 1: F(pv02[1]), 2: F(pv02[2]),
                3: F(pv02[3]), 5: F(q12a), 6: F(q12b),
                8: F(pv02[4]), 9: F(pv02[5]), 11: F(q13a), 12: F(q13b),
                14: F(pv02[6]), 15: F(pv02[7])})
            pv03 = pvs(0, 3, lambda: ex_h3, opj=True)
            pe_keepwarm(4)
            ex_g1 = scores_exp(1, 1, {
                0: F(pv03[0]), 2: F(pv03[1]), 4: F(pv03[2]), 6: F(pv03[3]),
                8: F(pv03[4]), 10: F(pv03[5]), 12: F(pv03[6]),
                14: F(pv03[7]),
                9: F(lambda: outproj_mm(0)), 11: F(lambda: outproj_mm(1)),
                13: F(lambda: outproj_mm(2))})
            pv10 = pvs(1, 0, lambda: ex_g0)
            pe_keepwarm(4)
            ex_g2 = scores_exp(1, 2, {
                0: F(pv10[0]), 2: F(pv10[1]), 4: F(pv10[2]), 6: F(pv10[3]),
                8: F(pv10[4]), 10: F(pv10[5]), 12: F(pv10[6]),
                14: F(pv10[7]),
                5: F(lambda: outproj_mm(3)), 7: F(lambda: outproj_mm(4)),
                9: F(lambda: outproj_mm(5)), 13: F(lambda: outproj_mm(6))})
            pv11 = pvs(1, 1, lambda: ex_g1)
            pv12 = pvs(1, 2, lambda: ex_g2)
            # final head split in two half-width passes so the first four
            # s-blocks' PV + out-projection overlap the second half's exps
            ex_g3a = scores_exp_half(1, 3, 0, {
                0: F(pv11[0]), 2: F(pv11[1]), 4: F(pv11[2]), 6: F(pv11[3]),
                8: F(pv11[4]), 10: F(pv11[5]), 12: F(pv11[6]),
                14: F(pv11[7]),
                3: F(pv12[0]), 7: F(pv12[1]), 11: F(pv12[2]),
                13: F(lambda: outproj_mm(7)), 15: F(pv12[3])})
            def pvh(qb):
                return lambda: pv_one(1, 3, ex_g3a, qb, with_atT=True,
                                      half=True)
            ex_g3b = scores_exp_half(1, 3, 1, {
                0: F(pv12[4]), 4: F(pv12[5]), 8: F(pv12[6]),
                12: F(pv12[7]),
                1: F(pvh(0)), 3: F(pvh(1)), 5: F(pvh(2)), 7: F(pvh(3)),
                9: F(lambda: outproj_mm(SBH)),
                11: F(lambda: outproj_mm(SBH + 1)),
                13: F(lambda: outproj_mm(SBH + 2)),
                15: F(lambda: outproj_mm(SBH + 3))})
            for qb in range(4, SBH):
                pv_one(1, 3, ex_g3b, qb, half=True, with_atT=True)
            pe_keepwarm(10)
            for qb in range(4, SBH):
                outproj_mm(SBH + qb)

    nc.compile()
    return nc


def shard_inputs(x, w_qkv, b_qkv, w_out, b_out, cfg: Cfg = FULL):
    """Build the 8 per-core input maps from full inputs (host-side layout
    marshaling: transpose/reshape/stack/dtype-cast, no arithmetic)."""
    DIN, DQK, KC, S = cfg.DIN, cfg.DQK, cfg.KC, cfg.S
    D = DIN
    bf16 = mybir.dt.np(mybir.dt.bfloat16)
    x = np.asarray(x, dtype=np.float32)
    w_qkv = np.asarray(w_qkv, dtype=np.float32)
    b_qkv = np.asarray(b_qkv, dtype=np.float32)
    w_out = np.asarray(w_out, dtype=np.float32)
    b_out = np.asarray(b_out, dtype=np.float32)
    zeros_bo = np.zeros((128, DIN), dtype=np.float32)
    bo128 = np.ascontiguousarray(
        np.broadcast_to(b_out.reshape(1, DIN), (128, DIN)))

    # x^T images per batch, as 2-s-block slabs:
    # row k2*128+p, col c*256+j = x[k2*256+j, c*128+p]
    xt_imgs = []
    for b in range(2):
        arr = x[b].astype(bf16).reshape(8, 256, KC, 128)  # (k2, j, c, p)
        xt_imgs.append(np.ascontiguousarray(
            arr.transpose(0, 3, 2, 1).reshape(8 * 128, S)))

    in_maps = []
    for c in range(cfg.N_CORES):
        b, hg = divmod(c, 4)
        sl = slice(hg * DQK, (hg + 1) * DQK)
        wq = w_qkv[0 * D:1 * D][sl]
        wk = w_qkv[1 * D:2 * D][sl]
        wv = w_qkv[2 * D:3 * D][sl]
        wo = w_out[:, sl]
        bq = b_qkv[0 * D:1 * D][sl]
        bk = b_qkv[1 * D:2 * D][sl]
        bqk_np = np.stack([bq[0:128], bq[128:256],
                           bk[0:128], bk[128:256]], axis=1)
        # w_qkT image [128, 4*1024]: col b*1024+c*128+j = W[b*128+j, c*128+p]
        wqk = np.concatenate([wq, wk], axis=0).astype(bf16)  # [512, 1024]
        wqkT = (wqk.reshape(4, 128, KC, 128)            # (blk, j, c, p)
                .transpose(3, 0, 2, 1).reshape(128, 4 * DIN))
        # w_vT image [128, 8*256]: col c*256+d = Wv[d, c*128+p]
        wvT = (wv.astype(bf16).reshape(DQK, KC, 128)    # (d, c, p)
               .transpose(2, 1, 0).reshape(128, KC * DQK))
        # w_oT image [128, 2*1024]: col ch*1024+dm = Wo[dm, ch*128+p]
        woT = (wo.astype(bf16).reshape(DIN, 2, 128)     # (dm, ch, p)
               .transpose(2, 1, 0).reshape(128, 2 * DIN))
        bv128 = np.broadcast_to(
            b_qkv[2 * D:3 * D][sl].reshape(1, DQK), (128, DQK))
        in_maps.append({
            "ident": np.eye(128, dtype=bf16),
            "xt": xt_imgs[b],
            "w_qkT": np.ascontiguousarray(wqkT),
            "w_vT": np.ascontiguousarray(wvT),
            "w_oT": np.ascontiguousarray(woT),
            "b_qk": np.ascontiguousarray(bqk_np),
            "b_v128": np.ascontiguousarray(bv128),
            "b_o128": bo128 if hg == 0 else zeros_bo,
        })
    return in_maps


def gather_output(results, cfg: Cfg = FULL):
    outs = []
    for b in range(2):
        acc = results[4 * b]["out_partial"].astype(np.float32)
        for c in range(4 * b + 1, 4 * b + 4):
            acc = acc + results[c]["out_partial"].astype(np.float32)
        outs.append(acc)
    return np.stack(outs, axis=0)


_NC_CACHE = {}


def _get_nc(cfg: Cfg = FULL):
    if cfg not in _NC_CACHE:
        _NC_CACHE[cfg] = build_nc(cfg)
    return _NC_CACHE[cfg]


def kernel(x, w_qkv, b_qkv, w_out, b_out):
    cfg = FULL
    nc = _get_nc(cfg)
    in_maps = shard_inputs(x, w_qkv, b_qkv, w_out, b_out, cfg)
    res = run_bass_kernel_spmd(nc, in_maps, core_ids=list(range(cfg.N_CORES)))
    return gather_output(res.results, cfg)


if __name__ == "__main__":
    rng = np.random.default_rng(0)
    D = FULL.DIN
    x = rng.standard_normal((2, FULL.S, D), dtype=np.float32)
    w_qkv = (rng.standard_normal((3 * D, D), dtype=np.float32) / np.sqrt(D))
    b_qkv = rng.standard_normal(3 * D, dtype=np.float32) * 0.02
    w_out = rng.standard_normal((D, D), dtype=np.float32) / np.sqrt(D)
    b_out = rng.standard_normal(D, dtype=np.float32) * 0.02
    out = kernel(x=x, w_qkv=w_qkv, b_qkv=b_qkv, w_out=w_out, b_out=b_out)
    print("out", out.shape, out.dtype, float(np.abs(out).mean()))


# revision 38
# speedup vs baseline: 1.0289x; 1.0289x over previous
"""Trainium2 Bass kernel for 16-head MultiHeadAttention (B=2, S=2048, D=1024).

Sharding: 8 cores = 2 (batch) x 4 (head groups of 4 heads).
Each core computes qkv projection for its 4 heads, attention, and a partial
out-projection (TP over heads); host sums the 4 partials per batch element.

Host-side sharding marshals data into device-friendly layouts (x and weights
pre-transposed so the contraction dim lands on SBUF partitions; biases
pre-stacked). On-device pipeline (single dependency-streamed pass):
  - Q^T(first half)/K^T(first half) projected as soon as their x^T s-blocks
    arrive; softmax exp (the ACT-engine floor, ~133us) starts ~18us in.
  - Attention per (q-half, head): scores -> PSUM ping-pong, exp [128,1024]
    on ACT, PV in transposed orientation (out [q=128, dk+1]) with the softmax
    denominator via a ones-column in V'. PV of head h overlaps exp of h+1;
    remaining projections/PV/out-proj are slot-woven into the score streams
    to fill PE slack under the exp stream.
  - Normalization via per-partition reciprocal+tensor_scalar_mul; attention
    rows stream into at^T (DMA transpose) + out-projection + DMA per s-block.
"""

import sys
from dataclasses import dataclass

for _p in ("/opt/trn_rl_repo",):
    if _p not in sys.path:
        sys.path.insert(0, _p)

import numpy as np

import concourse.bass as bass  # noqa: E402,F401
import concourse.bacc as bacc  # noqa: E402
import concourse.tile as tile  # noqa: E402
from concourse import mybir  # noqa: E402
from concourse.bass_utils import run_bass_kernel_spmd  # noqa: E402

F32 = mybir.dt.float32
BF16 = mybir.dt.bfloat16
AF = mybir.ActivationFunctionType


@dataclass(frozen=True)
class Cfg:
    S: int = 2048      # sequence length
    DIN: int = 1024    # model dim
    HPC: int = 4       # heads per core
    DK: int = 64       # head dim
    N_CORES: int = 8

    @property
    def DQK(self):
        return self.HPC * self.DK  # 256 per-core q/k/v width

    @property
    def KC(self):
        return self.DIN // 128     # 8 contraction chunks

    @property
    def SB(self):
        return self.S // 128       # 16 sequence blocks


FULL = Cfg()


def build_nc(cfg: Cfg = FULL):
    S, DIN, HPC, DK = cfg.S, cfg.DIN, cfg.HPC, cfg.DK
    DQK, KC, SB = cfg.DQK, cfg.KC, cfg.SB
    QC = 1024                 # q-chunk width for attention passes
    SBH = QC // 128           # 8 s-blocks per pass
    SCALE_INV = 1.0 / float(np.sqrt(DK))
    V65 = DK + 1              # V' width per head (denominator ones col)

    nc = bacc.Bacc("TRN2", target_bir_lowering=False, debug=False,
                   num_devices=cfg.N_CORES)

    # x^T in bf16, delivered as 2-s-block slabs: row k2*128+p holds
    # x[k2*256+j, c*128+p] at col c*256+j (see shard_inputs) so each
    # [128,2048] DMA delivers all 8 c-chunks of two s-blocks contiguously.
    xt_d = nc.dram_tensor("xt", [8 * 128, S], BF16, kind="ExternalInput")
    wqkT_d = nc.dram_tensor("w_qkT", [128, 4 * DIN], BF16,
                            kind="ExternalInput")
    wvT_d = nc.dram_tensor("w_vT", [128, KC * DQK], BF16,
                           kind="ExternalInput")
    woT_d = nc.dram_tensor("w_oT", [128, 2 * DIN], BF16,
                           kind="ExternalInput")
    bqk_d = nc.dram_tensor("b_qk", [128, 4], F32, kind="ExternalInput")
    id_d = nc.dram_tensor("ident", [128, 128], BF16, kind="ExternalInput")
    bv_d = nc.dram_tensor("b_v128", [128, DQK], F32, kind="ExternalInput")
    bo_d = nc.dram_tensor("b_o128", [128, DIN], F32, kind="ExternalInput")
    out_d = nc.dram_tensor("out_partial", [S, DIN], BF16,
                           kind="ExternalOutput")

    with tile.TileContext(nc) as tc:
        with (
            tc.tile_pool(name="persist", bufs=1) as pp,
            tc.tile_pool(name="expp", bufs=48) as ep,         # exp outputs
            tc.tile_pool(name="recp", bufs=4) as rp,
            tc.tile_pool(name="outp", bufs=4) as op_,
            tc.tile_pool(name="ps_s", bufs=2, space="PSUM") as pss,
            tc.tile_pool(name="ps_v", bufs=2, space="PSUM") as ppv,
            tc.tile_pool(name="ps_o", bufs=2, space="PSUM") as pso,
        ):
            # ---- persistent SBUF tensors (all-bf16 data path) ----
            xt = pp.tile([128, KC * S], BF16, tag="xt")          # x^T  [c][s]
            wqkT = pp.tile([128, 4 * DIN], BF16, tag="wqkT")     # [blk][c][j]
            wvT = pp.tile([128, KC * DQK], BF16, tag="wvT")      # [c][dout]
            woT = pp.tile([128, 2 * DIN], BF16, tag="woT")       # [ch][dm]
            qk = pp.tile([128, 4 * S], BF16, tag="qk")           # q0,q1,k0,k1
            vv = pp.tile([128, SB * HPC * V65], BF16, tag="vv")  # V' blocks
            at = pp.tile([128, SB * DQK], BF16, tag="at")        # attn out
            atT = pp.tile([128, 2 * S], BF16, tag="atT")         # at^T
            bqk = pp.tile([128, 4], F32, tag="bqk")
            ident = pp.tile([128, 128], BF16, tag="ident")
            bv128 = pp.tile([128, DQK], F32, tag="bv128")
            bo128 = pp.tile([128, DIN], F32, tag="bo128")

            xt3 = xt[:].rearrange("p (c s) -> p c s", c=KC)
            wqkT3 = wqkT[:].rearrange("p (b c j) -> p b c j", b=4, c=KC)
            wvT3 = wvT[:].rearrange("p (c d) -> p c d", c=KC)
            woT3 = woT[:].rearrange("p (h d) -> p h d", h=2)
            qk3 = qk[:].rearrange("p (b s) -> p b s", b=4)
            vv4 = vv[:].rearrange("p (i h d) -> p i h d", i=SB, h=HPC)
            at3 = at[:].rearrange("p (i d) -> p i d", i=SB)
            atT3 = atT[:].rearrange("p (h s) -> p h s", h=2)

            # ---- PE warm-up: keep the tensor engine busy from t=0 so
            # the p-state ramp completes before the first real projection ----
            warm = pp.tile([128, 512], BF16, tag="warm")
            nc.vector.memset(warm[:], 0.0)
            for _ in range(16):
                psw = pss.tile([128, QC], F32, tag="pss")
                nc.tensor.matmul(psw[:, 0:512], warm[:, 0:128], warm[:],
                                 start=True, stop=True)

            # ---- small loads ----
            for i in range(SB):
                nc.vector.memset(vv4[:, i, :, DK:V65], 1.0)
            # (bv128/bo128 loaded after the bulk weights/x below)

            def pe_keepwarm(n):
                # dep-free matmuls that fill known PE stall windows so the
                # p-state ramp doesn't reset (post-stall matmuls run 2-3.7x
                # slower in the cost model)
                for _ in range(n):
                    psw = pss.tile([128, QC], F32, tag="pss")
                    nc.tensor.matmul(psw[:, 0:512], warm[:, 0:128], warm[:],
                                     start=True, stop=True)

            # ---- bulk loads, ordered for earliest first score ----
            def dma_xt(k2):
                # two s-blocks per DMA (keeps >=512B contiguity)
                nc.sync.dma_start(
                    xt3[:, :, k2 * 256:(k2 + 1) * 256],
                    xt_d.ap()[k2 * 128:(k2 + 1) * 128, :])

            def dma_wqkT(b):
                nc.sync.dma_start(wqkT[:, b * DIN:(b + 1) * DIN],
                                  wqkT_d.ap()[:, b * DIN:(b + 1) * DIN])

            dma_wqkT(0)
            dma_xt(0)
            dma_xt(1)
            nc.sync.dma_start(bqk[:], bqk_d.ap())
            dma_wqkT(2)
            for k2 in range(2, 8):
                dma_xt(k2)
            nc.sync.dma_start(wvT[:], wvT_d.ap())
            dma_wqkT(1)
            dma_wqkT(3)
            nc.sync.dma_start(bv128[:], bv_d.ap())
            nc.sync.dma_start(woT[:], woT_d.ap())
            nc.sync.dma_start(bo128[:], bo_d.ap())
            nc.sync.dma_start(ident[:], id_d.ap())

            # ---- projections (bf16 matmuls, bf16 outputs) ----
            def proj_qkT(dblk, sc):
                # Q^T/K^T block dblk over s columns [sc*512, (sc+1)*512)
                ps = pss.tile([128, QC], F32, tag="pss")
                for c in range(KC):
                    nc.tensor.matmul(
                        ps[:, 0:512],
                        wqkT3[:, dblk, c, :],
                        xt3[:, c, sc * 512:(sc + 1) * 512],
                        start=(c == 0), stop=(c == KC - 1))
                nc.vector.tensor_scalar_add(
                    qk3[:, dblk, sc * 512:(sc + 1) * 512],
                    ps[:, 0:512], bqk[:, dblk:dblk + 1])

            def proj_v(i):
                ps = ppv.tile([128, 512], F32, tag="ppv")
                for c in range(KC):
                    nc.tensor.matmul(
                        ps[:, 0:DQK],
                        xt3[:, c, i * 128:(i + 1) * 128],
                        wvT3[:, c, :],
                        start=(c == 0), stop=(c == KC - 1))
                nc.vector.tensor_add(
                    vv4[:, i, :, 0:DK],
                    ps[:, 0:DQK].rearrange("p (h d) -> p h d", h=HPC),
                    bv128[:].rearrange("p (h d) -> p h d", h=HPC))

            proj_qkT(0, 0)
            proj_qkT(2, 0)
            proj_qkT(0, 1)
            proj_qkT(2, 1)

            # ---- attention machinery ----
            def scores_exp(qc, h, fills):
                """scores+exp for head h over q cols [qc*QC, (qc+1)*QC).
                fills: {slot: [thunks]} woven after score block <slot>."""
                pr, hl = divmod(h, 2)
                qblk, kblk = pr, 2 + pr
                exs = []
                for i in range(SB):
                    ps = pss.tile([128, QC], F32, tag="pss")
                    for n2 in range(QC // 512):
                        nc.tensor.matmul(
                            ps[:, n2 * 512:(n2 + 1) * 512],
                            qk3[64 * hl:64 * hl + 64, kblk,
                                i * 128:(i + 1) * 128],
                            qk3[64 * hl:64 * hl + 64, qblk,
                                qc * QC + n2 * 512:qc * QC + (n2 + 1) * 512],
                            start=True, stop=True)
                    ex = ep.tile([128, QC], BF16, tag="ex")
                    nc.scalar.activation(ex[:], ps[:], AF.Exp,
                                         scale=SCALE_INV)
                    exs.append(ex)
                    for t in fills.get(i, []):
                        t()
                return exs

            def scores_exp_half(qc, h, n2, fills):
                """half-width (512 q cols) scores+exp pass, for splitting the
                final head so its PV/out-proj can start early."""
                pr, hl = divmod(h, 2)
                qblk, kblk = pr, 2 + pr
                exs = []
                for i in range(SB):
                    ps = pss.tile([128, QC], F32, tag="pss")
                    nc.tensor.matmul(
                        ps[:, 0:512],
                        qk3[64 * hl:64 * hl + 64, kblk,
                            i * 128:(i + 1) * 128],
                        qk3[64 * hl:64 * hl + 64, qblk,
                            qc * QC + n2 * 512:qc * QC + (n2 + 1) * 512],
                        start=True, stop=True)
                    ex = ep.tile([128, QC], BF16, tag="ex")
                    nc.scalar.activation(ex[:, 0:512], ps[:, 0:512], AF.Exp,
                                         scale=SCALE_INV)
                    exs.append(ex)
                    for t in fills.get(i, []):
                        t()
                return exs

            def pv_one(qc, h, exs, qb, with_atT=False, half=False,
                       pe_atT=False):
                # PV (transposed) + normalize for one q-block of head h.
                # half: exs are [128,512] half-width tiles; qb is local 0-3.
                sblk = qc * SBH + qb
                qoff = (qb % 4) * 128 if half else qb * 128
                po = ppv.tile([128, 512], F32, tag="ppv")
                for i in range(SB):
                    nc.tensor.matmul(
                        po[:, 0:V65],
                        exs[i][:, qoff:qoff + 128],
                        vv4[:, i, h, :],
                        start=(i == 0), stop=(i == SB - 1))
                rec = rp.tile([128, 1], F32, tag="rec")
                nc.vector.reciprocal(rec[:], po[:, DK:V65])
                nc.vector.tensor_scalar_mul(
                    at3[:, sblk, h * DK:(h + 1) * DK],
                    po[:, 0:DK], rec[:])
                if pe_atT:
                    # PE transpose of the at row-block (no DMA latency)
                    pst = pso.tile([128, 512], F32, tag="pso")
                    pstb = pst.bitcast(BF16)
                    for ch in range(2):
                        nc.tensor.matmul(
                            pstb[:, ch * 128:(ch + 1) * 128],
                            at3[:, sblk, ch * 128:(ch + 1) * 128],
                            ident[:], is_transpose=True,
                            start=(ch == 0), stop=(ch == 1),
                            skip_group_check=True)
                    nc.vector.tensor_copy(
                        atT3[:, :, sblk * 128:(sblk + 1) * 128],
                        pstb[:, 0:256].rearrange("p (h j) -> p h j", h=2))
                elif with_atT:
                    nc.sync.dma_start_transpose(
                        atT3[:, :, sblk * 128:(sblk + 1) * 128],
                        at3[:, sblk, :])

            def outproj_mm(sblk):
                # at^T for sblk must already be in flight (pv_one with_atT)
                ot = op_.tile([128, DIN], BF16, tag="ot")
                for dmh in range(2):
                    ps = pso.tile([128, 512], F32, tag="pso")
                    for ch in range(2):
                        nc.tensor.matmul(
                            ps[:],
                            atT3[:, ch, sblk * 128:(sblk + 1) * 128],
                            woT3[:, ch, dmh * 512:(dmh + 1) * 512],
                            start=(ch == 0), stop=(ch == 1))
                    nc.vector.tensor_add(
                        ot[:, dmh * 512:(dmh + 1) * 512], ps[:],
                        bo128[:, dmh * 512:(dmh + 1) * 512])
                nc.sync.dma_start(
                    out_d.ap()[sblk * 128:(sblk + 1) * 128, :], ot[:])

            def F(*thunks):
                return list(thunks)

            qkp = (lambda d, sc: (lambda: proj_qkT(d, sc)))

            def qkp2(dblk, sc):
                # proj_qkT split into two 4-chunk thunks (finer PE weaving)
                box = []

                def first():
                    ps = pss.tile([128, QC], F32, tag="pss")
                    box.append(ps)
                    for c in range(4):
                        nc.tensor.matmul(
                            ps[:, 0:512],
                            wqkT3[:, dblk, c, :],
                            xt3[:, c, sc * 512:(sc + 1) * 512],
                            start=(c == 0), stop=False)

                def second():
                    ps = box[0]
                    for c in range(4, KC):
                        nc.tensor.matmul(
                            ps[:, 0:512],
                            wqkT3[:, dblk, c, :],
                            xt3[:, c, sc * 512:(sc + 1) * 512],
                            start=False, stop=(c == KC - 1))
                    nc.vector.tensor_scalar_add(
                        qk3[:, dblk, sc * 512:(sc + 1) * 512],
                        ps[:, 0:512], bqk[:, dblk:dblk + 1])
                return first, second
            vp = (lambda i: (lambda: proj_v(i)))

            def pvs(qc, h, exs_ref, opj=False):
                # one thunk per q-block; exs_ref resolved lazily by name
                def mk(qb):
                    def t():
                        pv_one(qc, h, exs_ref(), qb, with_atT=opj)
                        if opj and qb > 0:
                            outproj_mm(qc * SBH + qb - 1)
                    return t
                return [mk(qb) for qb in range(SBH)]

            # qc0
            k22a, k22b = qkp2(2, 2)
            k23a, k23b = qkp2(2, 3)
            ex_h0 = scores_exp(0, 0, {
                4: F(k22a), 5: F(k22b), 7: F(k23a), 8: F(k23b),
                9: F(vp(0)), 10: F(vp(1)), 11: F(vp(2)), 12: F(vp(3)),
                13: F(vp(4)), 14: F(vp(5)), 15: F(vp(6))})
            q10a, q10b = qkp2(1, 0)
            q11a, q11b = qkp2(1, 1)
            k30a, k30b = qkp2(3, 0)
            k31a, k31b = qkp2(3, 1)
            k32a, k32b = qkp2(3, 2)
            k33a, k33b = qkp2(3, 3)
            ex_h1 = scores_exp(0, 1, {
                0: F(q10a), 1: F(q10b), 3: F(q11a), 4: F(q11b),
                6: F(k30a), 7: F(k30b), 9: F(k31a), 10: F(k31b),
                12: F(k32a), 13: F(k32b), 14: F(k33a), 15: F(k33b)})
            pv00 = pvs(0, 0, lambda: ex_h0)
            ex_h2 = scores_exp(0, 2, {
                0: F(vp(7)), 1: F(vp(8)), 2: F(vp(9)), 3: F(vp(10)),
                4: F(vp(11)), 5: F(vp(12)), 6: F(vp(13)), 7: F(vp(14)),
                8: F(vp(15)),
                9: F(pv00[0]), 10: F(pv00[1]), 11: F(pv00[2]),
                12: F(pv00[3]), 13: F(pv00[4]), 14: F(pv00[5]),
                15: F(pv00[6], pv00[7])})
            q02a, q02b = qkp2(0, 2)
            q03a, q03b = qkp2(0, 3)
            pv01 = pvs(0, 1, lambda: ex_h1)
            ex_h3 = scores_exp(0, 3, {
                0: F(q02a), 1: F(q02b), 4: F(q03a), 5: F(q03b),
                2: F(pv01[0]), 3: F(pv01[1]), 6: F(pv01[2]), 7: F(pv01[3]),
                9: F(pv01[4]), 11: F(pv01[5]), 13: F(pv01[6]),
                15: F(pv01[7])})
            # qc1 (finishes qc0's PV + out-projection along the way)
            q12a, q12b = qkp2(1, 2)
            q13a, q13b = qkp2(1, 3)
            pv02 = pvs(0, 2, lambda: ex_h2)
            pe_keepwarm(4)
            ex_g0 = scores_exp(1, 0, {
                0: F(pv02[0]), 1: F(pv02[1]), 2: F(pv02[2]),
                3: F(pv02[3]), 5: F(q12a), 6: F(q12b),
                8: F(pv02[4]), 9: F(pv02[5]), 11: F(q13a), 12: F(q13b),
                14: F(pv02[6]), 15: F(pv02[7])})
            pv03 = pvs(0, 3, lambda: ex_h3, opj=True)
            pe_keepwarm(4)
            ex_g1 = scores_exp(1, 1, {
                0: F(pv03[0]), 2: F(pv03[1]), 4: F(pv03[2]), 6: F(pv03[3]),
                8: F(pv03[4]), 10: F(pv03[5]), 12: F(pv03[6]),
                14: F(pv03[7]),
                9: F(lambda: outproj_mm(0)), 11: F(lambda: outproj_mm(1)),
                13: F(lambda: outproj_mm(2))})
            pv10 = pvs(1, 0, lambda: ex_g0)
            pe_keepwarm(4)
            ex_g2 = scores_exp(1, 2, {
                0: F(pv10[0]), 2: F(pv10[1]), 4: F(pv10[2]), 6: F(pv10[3]),
                8: F(pv10[4]), 10: F(pv10[5]), 12: F(pv10[6]),
                14: F(pv10[7]),
                5: F(lambda: outproj_mm(3)), 7: F(lambda: outproj_mm(4)),
                9: F(lambda: outproj_mm(5)), 13: F(lambda: outproj_mm(6))})
            pv11 = pvs(1, 1, lambda: ex_g1)
            pv12 = pvs(1, 2, lambda: ex_g2)
            # final head split in two half-width passes so the first four
            # s-blocks' PV + out-projection overlap the second half's exps
            ex_g3a = scores_exp_half(1, 3, 0, {
                0: F(pv11[0]), 2: F(pv11[1]), 4: F(pv11[2]), 6: F(pv11[3]),
                8: F(pv11[4]), 10: F(pv11[5]), 12: F(pv11[6]),
                14: F(pv11[7]),
                3: F(pv12[0]), 7: F(pv12[1]), 11: F(pv12[2]),
                13: F(lambda: outproj_mm(7)), 15: F(pv12[3])})
            def pvh(qb):
                return lambda: pv_one(1, 3, ex_g3a, qb, with_atT=True,
                                      half=True)
            ex_g3b = scores_exp_half(1, 3, 1, {
                0: F(pv12[4]), 4: F(pv12[5]), 8: F(pv12[6]),
                12: F(pv12[7]),
                1: F(pvh(0)), 3: F(pvh(1)), 5: F(pvh(2)), 7: F(pvh(3)),
                9: F(lambda: outproj_mm(SBH)),
                11: F(lambda: outproj_mm(SBH + 1)),
                13: F(lambda: outproj_mm(SBH + 2)),
                15: F(lambda: outproj_mm(SBH + 3))})
            for qb in range(4, SBH):
                pv_one(1, 3, ex_g3b, qb, half=True, with_atT=True)
            pe_keepwarm(10)
            for qb in range(4, SBH):
                outproj_mm(SBH + qb)

    nc.compile()
    return nc


def shard_inputs(x, w_qkv, b_qkv, w_out, b_out, cfg: Cfg = FULL):
    """Build the 8 per-core input maps from full inputs (host-side layout
    marshaling: transpose/reshape/stack/dtype-cast, no arithmetic)."""
    DIN, DQK, KC, S = cfg.DIN, cfg.DQK, cfg.KC, cfg.S
    D = DIN
    bf16 = mybir.dt.np(mybir.dt.bfloat16)
    x = np.asarray(x, dtype=np.float32)
    w_qkv = np.asarray(w_qkv, dtype=np.float32)
    b_qkv = np.asarray(b_qkv, dtype=np.float32)
    w_out = np.asarray(w_out, dtype=np.float32)
    b_out = np.asarray(b_out, dtype=np.float32)
    zeros_bo = np.zeros((128, DIN), dtype=np.float32)
    bo128 = np.ascontiguousarray(
        np.broadcast_to(b_out.reshape(1, DIN), (128, DIN)))

    # x^T images per batch, as 2-s-block slabs:
    # row k2*128+p, col c*256+j = x[k2*256+j, c*128+p]
    xt_imgs = []
    for b in range(2):
        arr = x[b].astype(bf16).reshape(8, 256, KC, 128)  # (k2, j, c, p)
        xt_imgs.append(np.ascontiguousarray(
            arr.transpose(0, 3, 2, 1).reshape(8 * 128, S)))

    in_maps = []
    for c in range(cfg.N_CORES):
        b, hg = divmod(c, 4)
        sl = slice(hg * DQK, (hg + 1) * DQK)
        wq = w_qkv[0 * D:1 * D][sl]
        wk = w_qkv[1 * D:2 * D][sl]
        wv = w_qkv[2 * D:3 * D][sl]
        wo = w_out[:, sl]
        bq = b_qkv[0 * D:1 * D][sl]
        bk = b_qkv[1 * D:2 * D][sl]
        bqk_np = np.stack([bq[0:128], bq[128:256],
                           bk[0:128], bk[128:256]], axis=1)
        # w_qkT image [128, 4*1024]: col b*1024+c*128+j = W[b*128+j, c*128+p]
        wqk = np.concatenate([wq, wk], axis=0).astype(bf16)  # [512, 1024]
        wqkT = (wqk.reshape(4, 128, KC, 128)            # (blk, j, c, p)
                .transpose(3, 0, 2, 1).reshape(128, 4 * DIN))
        # w_vT image [128, 8*256]: col c*256+d = Wv[d, c*128+p]
        wvT = (wv.astype(bf16).reshape(DQK, KC, 128)    # (d, c, p)
               .transpose(2, 1, 0).reshape(128, KC * DQK))
        # w_oT image [128, 2*1024]: col ch*1024+dm = Wo[dm, ch*128+p]
        woT = (wo.astype(bf16).reshape(DIN, 2, 128)     # (dm, ch, p)
               .transpose(2, 1, 0).reshape(128, 2 * DIN))
        bv128 = np.broadcast_to(
            b_qkv[2 * D:3 * D][sl].reshape(1, DQK), (128, DQK))
        in_maps.append({
            "ident": np.eye(128, dtype=bf16),
            "xt": xt_imgs[b],
            "w_qkT": np.ascontiguousarray(wqkT),
            "w_vT": np.ascontiguousarray(wvT),
            "w_oT": np.ascontiguousarray(woT),
            "b_qk": np.ascontiguousarray(bqk_np),
            "b_v128": np.ascontiguousarray(bv128),
            "b_o128": bo128 if hg == 0 else zeros_bo,
        })
    return in_maps


def gather_output(results, cfg: Cfg = FULL):
    outs = []
    for b in range(2):
        acc = results[4 * b]["out_partial"].astype(np.float32)
        for c in range(4 * b + 1, 4 * b + 4):
            acc = acc + results[c]["out_partial"].astype(np.float32)
        outs.append(acc)
    return np.stack(outs, axis=0)


_NC_CACHE = {}


def _get_nc(cfg: Cfg = FULL):
    if cfg not in _NC_CACHE:
        _NC_CACHE[cfg] = build_nc(cfg)
    return _NC_CACHE[cfg]


def kernel(x, w_qkv, b_qkv, w_out, b_out):
    cfg = FULL
    nc = _get_nc(cfg)
    in_maps = shard_inputs(x, w_qkv, b_qkv, w_out, b_out, cfg)
    res = run_bass_kernel_spmd(nc, in_maps, core_ids=list(range(cfg.N_CORES)))
    return gather_output(res.results, cfg)


if __name__ == "__main__":
    rng = np.random.default_rng(0)
    D = FULL.DIN
    x = rng.standard_normal((2, FULL.S, D), dtype=np.float32)
    w_qkv = (rng.standard_normal((3 * D, D), dtype=np.float32) / np.sqrt(D))
    b_qkv = rng.standard_normal(3 * D, dtype=np.float32) * 0.02
    w_out = rng.standard_normal((D, D), dtype=np.float32) / np.sqrt(D)
    b_out = rng.standard_normal(D, dtype=np.float32) * 0.02
    out = kernel(x=x, w_qkv=w_qkv, b_qkv=b_qkv, w_out=w_out, b_out=b_out)
    print("out", out.shape, out.dtype, float(np.abs(out).mean()))
